# revision 5
# baseline (speedup 1.0000x reference)
"""Trainium2 Bass kernel for nn_MultiHeadAttention (B=2, T=2048, C=1024, H=16).

Sharding: 8 NeuronCores = 2 batch groups x 4 tensor-parallel cores.
Core c handles batch b = c // 4 and heads h0..h0+3, h0 = (c % 4) * 4.
Each core computes qkv projection for its head slice, causal attention for
its 4 heads, and a partial output projection (rows of W_out for its heads).
Host glue: slice weights per core, sum the 4 TP partials per batch, add b_out.

Device-side dataflow (all matmuls in bf16, fp32 accumulation):
  A: x [T,C] f32 --cast DMA--> xbf [T,C] bf16 (DRAM) --DMA transpose--> xT [C,T]
  B: qkT [512,T] = W_qk^T @ x^T (+bias)        (q,k head-major)
  C: V   [T,260] = x @ W_v (+bias, ones col)   (t-natural, 65 cols/head)
  D: per head: S^T[j,i] = K^T_j^T Q^T_i ; P = exp(S^T/8) * causal_mask;
     O^T|colsum = (V|1)^T @ P  (PSUM accum over j); O^T /= colsum
  E: y_partial [T, C] = (O^T)^T @ W_out_rows
"""

import sys

sys.path.insert(0, "/opt/trn_rl_repo")

import numpy as np
import ml_dtypes

import concourse.bass as bass
import concourse.mybir as mybir
from concourse.tile import TileContext
from concourse.bass_utils import run_bass_kernel_spmd

T = 2048
C = 1024
H = 16
D = 64
NCORE = 8
TPG = 4          # tensor-parallel group size (cores per batch)
HC = H // TPG    # heads per core
CL = HC * D      # local c dim (256)
F32 = mybir.dt.float32
BF16 = mybir.dt.bfloat16
AF = mybir.ActivationFunctionType


def _build_program():
    nc = bass.Bass("TRN2", target_bir_lowering=False, debug=False)

    x = nc.declare_dram_parameter("x", [T, C], F32, isOutput=False)
    wqk = nc.declare_dram_parameter("wqk", [C, 2 * CL], F32, isOutput=False)
    bqk = nc.declare_dram_parameter("bqk", [2 * CL], F32, isOutput=False)
    wv = nc.declare_dram_parameter("wv", [C, CL], F32, isOutput=False)
    bv = nc.declare_dram_parameter("bv", [CL], F32, isOutput=False)
    wo = nc.declare_dram_parameter("wo", [CL, C], F32, isOutput=False)
    trimask = nc.declare_dram_parameter("trimask", [128, 128], BF16, isOutput=False)
    y = nc.declare_dram_parameter("y", [T, C], F32, isOutput=True)

    NT = T // 128     # 16 t-tiles
    NCB = C // 128    # 8 c-tiles
    NIC = T // 512    # 4 512-query chunks

    with TileContext(nc) as tc:
        with (
            tc.tile_pool(name="singles", bufs=1) as singles,
            tc.tile_pool(name="xstage", bufs=4) as xstage,
            tc.tile_pool(name="ptp", bufs=3) as ptp,
            tc.tile_pool(name="small", bufs=4) as small,
            tc.tile_pool(name="yout", bufs=4) as yout,
            tc.tile_pool(name="dram", bufs=1, space="DRAM") as dram,
            tc.tile_pool(name="psum_big", bufs=2, space="PSUM") as pp_big,
            tc.tile_pool(name="psum_ot", bufs=2, space="PSUM") as pp_ot,
            tc.tile_pool(name="psum_bc", bufs=2, space="PSUM") as pp_bc,
        ):
            # ---- persistent SBUF tensors ----
            xT = singles.tile([128, NCB, T], BF16)        # x^T, c on partitions
            wqk_sb = singles.tile([128, NCB, 2 * CL], BF16)
            wv_sb = singles.tile([128, NCB, CL], BF16)
            wo_sb = singles.tile([128, 2, C], BF16)
            qkT = singles.tile([128, 4, T], BF16)         # [q01,q23,k01,k23]
            v_sb = singles.tile([128, NT, HC, D + 1], BF16)
            ot_sb = singles.tile([128, 2, T], BF16)       # O^T, c_local on part
            bqk_sb = singles.tile([128, 4], F32)
            bv_sb = singles.tile([128, CL], F32)
            mask_sb = singles.tile([128, 128], BF16)
            ones_sb = singles.tile([1, 64], F32)

            xbf = dram.tile([T, C], BF16)

            # ---- phase A: stage/cast x, transpose; load weights ----
            nc.vector.memset(ones_sb, 1.0)
            nc.sync.dma_start(out=mask_sb, in_=trimask[:, :])
            for m in range(4):
                nc.sync.dma_start(
                    out=bqk_sb[:, m : m + 1], in_=bqk[m * 128 : (m + 1) * 128, None]
                )
            nc.gpsimd.dma_start(out=bv_sb, in_=bv[None, :].to_broadcast((128, CL)))
            for kc in range(NCB):
                nc.gpsimd.dma_start(
                    out=wqk_sb[:, kc, :], in_=wqk[kc * 128 : (kc + 1) * 128, :]
                )
                nc.gpsimd.dma_start(
                    out=wv_sb[:, kc, :], in_=wv[kc * 128 : (kc + 1) * 128, :]
                )
            for kc in range(2):
                nc.gpsimd.dma_start(
                    out=wo_sb[:, kc, :], in_=wo[kc * 128 : (kc + 1) * 128, :]
                )

            for tt in range(NT):
                xs = xstage.tile([128, C], BF16)
                nc.gpsimd.dma_start(out=xs, in_=x[tt * 128 : (tt + 1) * 128, :])
                nc.sync.dma_start(out=xbf[tt * 128 : (tt + 1) * 128, :], in_=xs)
            for tch in range(NIC):
                for cb in range(NCB):
                    nc.sync.dma_start(
                        out=xT[:, cb, tch * 512 : (tch + 1) * 512],
                        in_=xbf[tch * 512 : (tch + 1) * 512, cb * 128 : (cb + 1) * 128],
                        transpose=True,
                    )

            # ---- phases B + C: qk^T projection and V (natural) ----
            for ic in range(NIC):
                for m in range(4):
                    ps = pp_big.tile([128, 1024], F32, tag="big")
                    for kc in range(NCB):
                        nc.tensor.matmul(
                            ps[:, 0:512],
                            lhsT=wqk_sb[:, kc, m * 128 : (m + 1) * 128],
                            rhs=xT[:, kc, ic * 512 : (ic + 1) * 512],
                            start=(kc == 0),
                            stop=(kc == NCB - 1),
                        )
                    nc.vector.tensor_scalar_add(
                        out=qkT[:, m, ic * 512 : (ic + 1) * 512],
                        in0=ps[:, 0:512],
                        scalar1=bqk_sb[:, m : m + 1],
                    )
                for tt in range(4 * ic, 4 * ic + 4):
                    ps = pp_big.tile([128, 1024], F32, tag="big")
                    for kc in range(NCB):
                        nc.tensor.matmul(
                            ps[:, 0:CL],
                            lhsT=xT[:, kc, tt * 128 : (tt + 1) * 128],
                            rhs=wv_sb[:, kc, :],
                            start=(kc == 0),
                            stop=(kc == NCB - 1),
                        )
                    nc.vector.tensor_tensor(
                        out=v_sb[:, tt, :, 0:D],
                        in0=ps[:, 0:CL].rearrange("p (h d) -> p h d", h=HC),
                        in1=bv_sb.rearrange("p (h d) -> p h d", h=HC),
                        op=mybir.AluOpType.add,
                    )
                    nc.vector.memset(v_sb[:, tt, :, D : D + 1], 1.0)

            # ---- phase D: attention per 1024-wide query chunk, per head ----
            for ic2 in range(2):
                c0 = ic2 * 1024
                n_jt = 8 * (ic2 + 1)
                for h in range(HC):
                    pb = (h % 2) * 64
                    qt = qkT[pb : pb + 64, h // 2, :]
                    kt = qkT[pb : pb + 64, 2 + h // 2, :]
                    ots = [pp_ot.tile([65, 512], F32, tag="ot", name=f"ot_{ic2}_{h}_{i}") for i in range(2)]
                    for jt in range(n_jt):
                        off = max(0, jt * 128 - c0)
                        st = pp_big.tile([128, 1024], F32, tag="big")
                        pt = ptp.tile([128, 1024], BF16)
                        for sc in range(2):
                            lo = sc * 512
                            if lo + 512 <= off:
                                continue
                            nc.tensor.matmul(
                                st[:, lo : lo + 512],
                                lhsT=kt[:, jt * 128 : (jt + 1) * 128],
                                rhs=qt[:, c0 + lo : c0 + lo + 512],
                                start=True,
                                stop=True,
                            )
                        nc.scalar.activation(
                            out=pt[:, off:1024],
                            in_=st[:, off:1024],
                            func=AF.Exp,
                            scale=0.125,
                        )
                        if jt * 128 >= c0:
                            nc.vector.tensor_mul(
                                pt[:, off : off + 128],
                                pt[:, off : off + 128],
                                mask_sb,
                            )
                        for sc in range(2):
                            lo = sc * 512
                            a = max(off, lo)
                            if a >= lo + 512:
                                continue
                            last_jt = (8 * ic2 + 4 * sc + 4) - 1
                            nc.tensor.matmul(
                                ots[sc][:, a - lo : 512],
                                lhsT=v_sb[:, jt, h, :],
                                rhs=pt[:, a : lo + 512],
                                start=(jt == 0),
                                stop=(jt == last_jt),
                            )
                    for sc in range(2):
                        rec = small.tile([1, 512], F32, tag="rec")
                        nc.vector.reciprocal(rec, ots[sc][64:65, :])
                        bc_ps = pp_bc.tile([64, 512], F32, tag="bc")
                        nc.tensor.matmul(
                            bc_ps, lhsT=ones_sb, rhs=rec, start=True, stop=True
                        )
                        bc_sb = small.tile([64, 512], F32, tag="bcs")
                        nc.vector.tensor_copy(bc_sb, bc_ps)
                        nc.vector.tensor_mul(
                            ot_sb[pb : pb + 64, h // 2, c0 + sc * 512 : c0 + (sc + 1) * 512],
                            ots[sc][0:64, :],
                            bc_sb,
                        )

            # ---- phase E: partial out-projection ----
            for tt in range(NT):
                for nch in range(2):
                    ps = pp_big.tile([128, 1024], F32, tag="big")
                    for kc in range(2):
                        nc.tensor.matmul(
                            ps[:, 0:512],
                            lhsT=ot_sb[:, kc, tt * 128 : (tt + 1) * 128],
                            rhs=wo_sb[:, kc, nch * 512 : (nch + 1) * 512],
                            start=(kc == 0),
                            stop=(kc == 1),
                        )
                    ys = yout.tile([128, 512], F32)
                    nc.vector.tensor_copy(ys, ps[:, 0:512])
                    nc.sync.dma_start(
                        out=y[tt * 128 : (tt + 1) * 128, nch * 512 : (nch + 1) * 512],
                        in_=ys,
                    )

    _split_multi_waits(nc)
    return nc


_WAIT_CTR = [0]


def _split_multi_waits(nc, max_waits=1):
    """This container's walrus accepts only ONE sem wait per instruction.
    Hoist extra waits onto standalone EventSemaphore insts just before."""
    for f in nc.m.functions:
        for bb in f.blocks:
            insts = list(bb.instructions)
            out = []
            changed = False
            for inst in insts:
                si = inst.sync_info
                if si is not None and len(si.on_wait) > max_waits:
                    waits = list(si.on_wait)
                    keep, extra = waits[-max_waits:], waits[:-max_waits]
                    for w in extra:
                        _WAIT_CTR[0] += 1
                        out.append(
                            mybir.InstEventSemaphore(
                                name=f"xw-{_WAIT_CTR[0]}",
                                engine=inst.engine,
                                ins=[],
                                outs=[],
                                sync_info=mybir.SyncInfo(on_wait=[w], on_update=[]),
                            )
                        )
                    inst.sync_info = mybir.SyncInfo(
                        on_wait=keep, on_update=list(si.on_update)
                    )
                    changed = True
                out.append(inst)
            if changed:
                bb.instructions = out


_PROGRAM = None


def _get_program():
    global _PROGRAM
    if _PROGRAM is None:
        _PROGRAM = _build_program()
    return _PROGRAM


def _make_in_maps(x, W_attn, b_attn, W_out, b_out):
    bf16 = ml_dtypes.bfloat16
    tri = np.triu(np.ones((128, 128), dtype=bf16))  # mask[j, i] = j <= i
    in_maps = []
    for core in range(NCORE):
        b = core // TPG
        h0 = (core % TPG) * HC
        qcols = slice(h0 * D, (h0 + HC) * D)
        kcols = slice(C + h0 * D, C + (h0 + HC) * D)
        vcols = slice(2 * C + h0 * D, 2 * C + (h0 + HC) * D)
        in_maps.append(
            {
                "x": np.ascontiguousarray(x[b]),
                "wqk": np.ascontiguousarray(
                    np.concatenate([W_attn[:, qcols], W_attn[:, kcols]], axis=1)
                ),
                "bqk": np.ascontiguousarray(
                    np.concatenate([b_attn[qcols], b_attn[kcols]])
                ),
                "wv": np.ascontiguousarray(W_attn[:, vcols]),
                "bv": np.ascontiguousarray(b_attn[vcols]),
                "wo": np.ascontiguousarray(W_out[h0 * D : (h0 + HC) * D, :]),
                "trimask": tri,
            }
        )
    return in_maps


def _run(x, W_attn, b_attn, W_out, b_out, trace=False):
    nc = _get_program()
    in_maps = _make_in_maps(x, W_attn, b_attn, W_out, b_out)
    res = run_bass_kernel_spmd(nc, in_maps, list(range(NCORE)), trace=trace)
    parts = [res.results[i]["y"].astype(np.float32) for i in range(NCORE)]
    out = np.stack(
        [
            parts[0] + parts[1] + parts[2] + parts[3],
            parts[4] + parts[5] + parts[6] + parts[7],
        ]
    )
    out += b_out.astype(np.float32)
    return out, res


def kernel(x, W_attn, b_attn, W_out, b_out):
    out, _ = _run(
        np.asarray(x), np.asarray(W_attn), np.asarray(b_attn),
        np.asarray(W_out), np.asarray(b_out),
    )
    return out
